# revision 28
# baseline (speedup 1.0000x reference)
"""Trainium2 Bass kernel for nn_DLCF_DCA (scatter_memory).

Reference computation, per sample b (B=128, S=256, H=768, K=64):
  keep_dep[s]  = (s==0) or any_k(depend[b,k] == s-1)
  keep_dpd[s]  = (s==0) or any_k(depended[b,k] == s-1)
  mult[s]      = w2 if s-1 in depended else (w1 if s-1 in depend else 0);
                 0 if s-1 in no_connect; 1 if s==0
  y1 = x * keep_dep;  y2 = x * keep_dpd;  y3 = x * mult

Strategy: pure data parallel over batch (16 samples per core, 8 cores).
The tiny per-token multiplier tables ([B, S] = 32K floats total) are
assembled on the host from the index lists; the device does the pure
memory-bound work: stream the [4096, 768] bf16 shard in (32 consecutive
token-rows per SBUF partition, so every DMA moves 6KB contiguous chunks
per partition), apply the three per-row scalars on the vector engine,
and stream the three outputs back out on three DMA queues (sync /
scalar / gpsimd) so all 16 SDMA engines stay saturated end to end.
"""

import contextlib
import os
import sys

import numpy as np

if "/opt/trn_rl_repo" not in sys.path:
    sys.path.insert(0, "/opt/trn_rl_repo")

N_CORES = 8
B, S, H, K = 128, 256, 768, 64
BL = B // N_CORES          # samples per core
ROWS = BL * S              # 4096 token-rows per core
RPP = ROWS // 128          # 32 consecutive rows per partition
NDR = 8                    # read DMA tiles (6KB/partition; completions stagger)
RPTR = RPP // NDR          # 4 row-blocks per read tile
NDW = 4                    # write DMA tiles per output (12KB/partition)
RPTW = RPP // NDW          # 8 row-blocks per write tile

_cache = {}


def _split_multiwaits(nc, max_waits=1):
    """walrus in this container only accepts one sync-wait per instruction;
    splice extra waits onto single-wait NoOps just before the offender."""
    from concourse import mybir

    n = 0
    for func in nc.m.functions:
        for bb in func.blocks:
            insts = bb.instructions
            i = 0
            while i < len(insts):
                ins = insts[i]
                si = getattr(ins, "sync_info", None)
                if si is None or len(si.on_wait) <= max_waits:
                    i += 1
                    continue
                waits = list(si.on_wait)
                keep = waits[-max_waits:]
                extra = waits[:-max_waits]
                nops = []
                for j in range(0, len(extra), max_waits):
                    n += 1
                    nops.append(
                        mybir.InstNoOp(
                            name=f"{ins.name}-ws{n}",
                            sync_info=mybir.SyncInfo(
                                on_wait=extra[j : j + max_waits], on_update=[]
                            ),
                            bass_nofuse=True,
                            engine=ins.engine,
                            ins=[],
                            outs=[],
                        )
                    )
                si.on_wait = keep
                for k, nop in enumerate(nops):
                    insts.insert(i + k, nop)
                i += len(nops) + 1
    return n


def _build():
    import concourse.bass as bass
    import concourse.tile as tile
    from concourse import mybir

    f32 = mybir.dt.float32
    bf16 = mybir.dt.bfloat16
    mul = mybir.AluOpType.mult
    nc = bass.Bass()

    x = nc.dram_tensor("x", [ROWS, H], bf16, kind="ExternalInput")
    masks = nc.dram_tensor("masks", [128 * 3 * RPP], f32, kind="ExternalInput")
    ys = [nc.dram_tensor(f"y{i}", [ROWS, H], bf16, kind="ExternalOutput")
          for i in (1, 2, 3)]

    with tile.TileContext(nc) as tc, contextlib.ExitStack() as ctx:
        const = ctx.enter_context(tc.tile_pool(name="const", bufs=1))
        xpool = ctx.enter_context(tc.tile_pool(name="xpool", bufs=NDR))
        ypools = [
            ctx.enter_context(tc.tile_pool(name=f"y{i}p", bufs=4))
            for i in (1, 2, 3)
        ]

        # per-row multipliers, in [partition, row-in-partition] layout
        mt = const.tile([128, 3 * RPP], f32, name="masks")
        nc.sync.dma_start(out=mt[:], in_=masks.rearrange("(p c) -> p c", p=128))
        m = [mt[:, i * RPP : (i + 1) * RPP] for i in range(3)]

        # row = p*32 + d*RPT + g: partition p owns 32 consecutive token-rows.
        xr = x.rearrange("(p d q) h -> d p (q h)", p=128, d=NDR)
        yr = [y.rearrange("(p d q) h -> d p (q h)", p=128, d=NDW) for y in ys]

        # All reads on the sync queue: the scalar queue then has no read
        # backlog, so its writes can enter service the moment the gate fires
        # (a split-read layout forces BOTH queues into read-then-write FIFO
        # and re-creates a hard transition bubble).
        xts = []
        for d in range(NDR):
            t = xpool.tile([128, RPTR * H], bf16, name="xt")
            nc.sync.dma_start(out=t[:], in_=xr[d])
            xts.append(t)

        # Phase separation: pure-read burst, then pure-write burst, avoiding
        # the HBM read/write turnaround penalty (~13% per-engine throughput
        # when mixed). DVE computes freely as each x tile lands (same-queue
        # DMAs complete in FIFO order, so completions stagger). The gate
        # lives on the otherwise-idle ACT engine (gpsimd is locked out of
        # SBUF while DVE streams perf-mode ops; a DVE gate would serialize
        # all compute behind the reads): gone = x5*0.0 + 1.0 == exact 1.0,
        # dependent on the 5th of 8 reads — the last ~2.4MB of reads overlap
        # the first writes, hiding the DMA completion-receipt latency. Every
        # write tile then gets a tiny ACT "stamp" (*1.0, exact) that its
        # write issue waits on.
        copyf = mybir.ActivationFunctionType.Copy
        gone = const.tile([128, 1], f32, name="gone")
        nc.scalar.activation(gone[:], xts[3][:, :1], copyf, scale=0.0, bias=1.0)

        rings = [nc.scalar, nc.scalar, nc.sync]
        for d in range(NDW):
            for yi in range(3):
                yt = ypools[yi].tile([128, RPTW * H], bf16, name=f"y{yi}t")
                for g in range(RPTW):
                    r = d * RPTW + g
                    blk = slice(g * H, (g + 1) * H)
                    nc.vector.tensor_scalar(
                        yt[:, blk], xts[r // RPTR][:, (r % RPTR) * H : (r % RPTR + 1) * H],
                        m[yi][:, r : r + 1], None, op0=mul,
                    )
                nc.scalar.activation(yt[:, :1], yt[:, :1], copyf,
                                     scale=gone[:, 0:1])
                rings[yi].dma_start(out=yr[yi][d], in_=yt[:])

    _split_multiwaits(nc)
    return nc


def _prep_inputs(bert_local_out, depend, depended, no_connect,
                 depend_weight, depended_weight):
    import ml_dtypes

    x = np.ascontiguousarray(
        np.asarray(bert_local_out, dtype=np.float32).astype(ml_dtypes.bfloat16)
    )
    dep = np.asarray(depend, dtype=np.int64)
    dpd = np.asarray(depended, dtype=np.int64)
    noc = np.asarray(no_connect, dtype=np.int64)
    w1 = np.asarray(depend_weight, dtype=np.float32)
    w2 = np.asarray(depended_weight, dtype=np.float32)

    # Per-token multipliers, matching the reference's scatter order exactly.
    # Index lists hold values in [0, S); position idx+1 is affected (idx=-1
    # padding or idx=S-1 land in slots 0/S which are overwritten/cropped).
    rr = np.arange(B)[:, None]
    m1 = np.zeros((B, S + 1), np.float32)
    m1[rr, dep + 1] = 1.0
    m2 = np.zeros((B, S + 1), np.float32)
    m2[rr, dpd + 1] = 1.0
    m3 = np.zeros((B, S + 1), np.float32)
    m3[rr, dep + 1] = np.broadcast_to(w1[:, None], (B, K))
    m3[rr, dpd + 1] = np.broadcast_to(w2[:, None], (B, K))
    m3[rr, noc + 1] = 0.0
    for mm in (m1, m2, m3):
        mm[:, 0] = 1.0
    masks = np.stack([m1[:, :S], m2[:, :S], m3[:, :S]])  # [3, B, S]

    in_maps = []
    for c in range(N_CORES):
        sl = slice(c * BL, (c + 1) * BL)
        mc = masks[:, sl].reshape(3, 128, RPP)          # row = p*32 + r
        mc = np.ascontiguousarray(mc.transpose(1, 0, 2))  # [128, 3, RPP]
        in_maps.append({
            "x": x[sl].reshape(ROWS, H),
            "masks": mc.reshape(-1),
        })
    return in_maps


def kernel(bert_local_out, depend, depended, no_connect,
           depend_weight, depended_weight):
    from concourse.bass_utils import run_bass_kernel_spmd

    if "nc" not in _cache:
        _cache["nc"] = _build()
    nc = _cache["nc"]

    in_maps = _prep_inputs(bert_local_out, depend, depended, no_connect,
                           depend_weight, depended_weight)

    pdir = os.environ.get("KERNEL_PROFILE_DIR")
    ctx = contextlib.nullcontext()
    if pdir:
        import concourse.bass2jax as b2j
        from trn_agent_boot.trn_boot import _ntff_profile_via_ctypes

        if not getattr(b2j, "_neff_capture_patched", False):
            orig = b2j.rename_neff_tensors_and_patch_header

            def patched(neff_path, mapping):
                data = orig(neff_path, mapping)
                cap = os.environ.get("KERNEL_PROFILE_DIR")
                if cap:
                    os.makedirs(cap, exist_ok=True)
                    with open(os.path.join(cap, "model.neff"), "wb") as f:
                        f.write(data)
                return data

            b2j.rename_neff_tensors_and_patch_header = patched
            b2j._neff_capture_patched = True
        os.makedirs(pdir, exist_ok=True)
        hookf = _ntff_profile_via_ctypes("/opt/axon/libaxon_pjrt.so")
        if hookf is not None:
            dev = None if os.environ.get("KERNEL_PROFILE_ALL") else [0]
            ctx = hookf(pdir, dev)

    with ctx:
        res = run_bass_kernel_spmd(nc, in_maps, list(range(N_CORES)))

    outs = []
    for name in ("y1", "y2", "y3"):
        full = np.empty((B, S, H), dtype=np.float32)
        for c in range(N_CORES):
            full[c * BL : (c + 1) * BL] = (
                res.results[c][name].astype(np.float32).reshape(BL, S, H)
            )
        outs.append(full)
    return tuple(outs)


# revision 30
# speedup vs baseline: 1.1493x; 1.1493x over previous
"""Trainium2 Bass kernel for nn_DLCF_DCA (scatter_memory).

Reference computation, per sample b (B=128, S=256, H=768, K=64):
  keep_dep[s]  = (s==0) or any_k(depend[b,k] == s-1)
  keep_dpd[s]  = (s==0) or any_k(depended[b,k] == s-1)
  mult[s]      = w2 if s-1 in depended else (w1 if s-1 in depend else 0);
                 0 if s-1 in no_connect; 1 if s==0
  y1 = x * keep_dep;  y2 = x * keep_dpd;  y3 = x * mult

Strategy: pure data parallel over batch (16 samples per core, 8 cores).
The tiny per-token multiplier tables ([B, S] = 32K floats total) are
assembled on the host from the index lists; the device does the pure
memory-bound work: stream the [4096, 768] bf16 shard in (32 consecutive
token-rows per SBUF partition, so every DMA moves 6KB contiguous chunks
per partition), apply the three per-row scalars on the vector engine,
and stream the three outputs back out on three DMA queues (sync /
scalar / gpsimd) so all 16 SDMA engines stay saturated end to end.
"""

import contextlib
import os
import sys

import numpy as np

if "/opt/trn_rl_repo" not in sys.path:
    sys.path.insert(0, "/opt/trn_rl_repo")

N_CORES = 8
B, S, H, K = 128, 256, 768, 64
BL = B // N_CORES          # samples per core
ROWS = BL * S              # 4096 token-rows per core
RPP = ROWS // 128          # 32 consecutive rows per partition
NDR = 8                    # read DMA tiles (6KB/partition; completions stagger)
RPTR = RPP // NDR          # 4 row-blocks per read tile
NDW = 2                    # write DMA tiles per output (24KB/partition): only
                           # 3 DMAs per queue, below the HWDGE ring depth, so
                           # no write issue ever blocks on a prior completion
RPTW = RPP // NDW          # 16 row-blocks per write tile

_cache = {}


def _split_multiwaits(nc, max_waits=1):
    """walrus in this container only accepts one sync-wait per instruction;
    splice extra waits onto single-wait NoOps just before the offender."""
    from concourse import mybir

    n = 0
    for func in nc.m.functions:
        for bb in func.blocks:
            insts = bb.instructions
            i = 0
            while i < len(insts):
                ins = insts[i]
                si = getattr(ins, "sync_info", None)
                if si is None or len(si.on_wait) <= max_waits:
                    i += 1
                    continue
                waits = list(si.on_wait)
                keep = waits[-max_waits:]
                extra = waits[:-max_waits]
                nops = []
                for j in range(0, len(extra), max_waits):
                    n += 1
                    nops.append(
                        mybir.InstNoOp(
                            name=f"{ins.name}-ws{n}",
                            sync_info=mybir.SyncInfo(
                                on_wait=extra[j : j + max_waits], on_update=[]
                            ),
                            bass_nofuse=True,
                            engine=ins.engine,
                            ins=[],
                            outs=[],
                        )
                    )
                si.on_wait = keep
                for k, nop in enumerate(nops):
                    insts.insert(i + k, nop)
                i += len(nops) + 1
    return n


def _build():
    import concourse.bass as bass
    import concourse.tile as tile
    from concourse import mybir

    f32 = mybir.dt.float32
    bf16 = mybir.dt.bfloat16
    mul = mybir.AluOpType.mult
    nc = bass.Bass()

    x = nc.dram_tensor("x", [ROWS, H], bf16, kind="ExternalInput")
    masks = nc.dram_tensor("masks", [128 * 3 * RPP], f32, kind="ExternalInput")
    ys = [nc.dram_tensor(f"y{i}", [ROWS, H], bf16, kind="ExternalOutput")
          for i in (1, 2, 3)]

    with tile.TileContext(nc) as tc, contextlib.ExitStack() as ctx:
        const = ctx.enter_context(tc.tile_pool(name="const", bufs=1))
        xpool = ctx.enter_context(tc.tile_pool(name="xpool", bufs=NDR))
        ypools = [
            ctx.enter_context(tc.tile_pool(name=f"y{i}p", bufs=NDW))
            for i in (1, 2, 3)
        ]

        # per-row multipliers, in [partition, row-in-partition] layout
        mt = const.tile([128, 3 * RPP], f32, name="masks")
        nc.sync.dma_start(out=mt[:], in_=masks.rearrange("(p c) -> p c", p=128))
        m = [mt[:, i * RPP : (i + 1) * RPP] for i in range(3)]

        # row = p*32 + d*RPT + g: partition p owns 32 consecutive token-rows.
        xr = x.rearrange("(p d q) h -> d p (q h)", p=128, d=NDR)
        yr = [y.rearrange("(p d q) h -> d p (q h)", p=128, d=NDW) for y in ys]

        # All reads on the sync queue: the scalar queue then has no read
        # backlog, so its writes can enter service the moment the gate fires
        # (a split-read layout forces BOTH queues into read-then-write FIFO
        # and re-creates a hard transition bubble).
        xts = []
        for d in range(NDR):
            t = xpool.tile([128, RPTR * H], bf16, name="xt")
            nc.sync.dma_start(out=t[:], in_=xr[d])
            xts.append(t)

        # Phase separation: pure-read burst, then pure-write burst, avoiding
        # the HBM read/write turnaround penalty (~13% per-engine throughput
        # when mixed). DVE computes freely as each x tile lands (same-queue
        # DMAs complete in FIFO order, so completions stagger). The gate
        # lives on the otherwise-idle ACT engine (gpsimd is locked out of
        # SBUF while DVE streams perf-mode ops; a DVE gate would serialize
        # all compute behind the reads): gone = x5*0.0 + 1.0 == exact 1.0,
        # dependent on the 5th of 8 reads — the last ~2.4MB of reads overlap
        # the first writes, hiding the DMA completion-receipt latency. Every
        # write tile then gets a tiny ACT "stamp" (*1.0, exact) that its
        # write issue waits on.
        copyf = mybir.ActivationFunctionType.Copy
        gone = const.tile([128, 1], f32, name="gone")
        nc.scalar.activation(gone[:], xts[3][:, :1], copyf, scale=0.0, bias=1.0)

        rings = [nc.scalar, nc.scalar, nc.sync]
        for d in range(NDW):
            for yi in range(3):
                yt = ypools[yi].tile([128, RPTW * H], bf16, name=f"y{yi}t")
                for g in range(RPTW):
                    r = d * RPTW + g
                    blk = slice(g * H, (g + 1) * H)
                    nc.vector.tensor_scalar(
                        yt[:, blk], xts[r // RPTR][:, (r % RPTR) * H : (r % RPTR + 1) * H],
                        m[yi][:, r : r + 1], None, op0=mul,
                    )
                nc.scalar.activation(yt[:, :1], yt[:, :1], copyf,
                                     scale=gone[:, 0:1])
                rings[yi].dma_start(out=yr[yi][d], in_=yt[:])

    _split_multiwaits(nc)
    return nc


def _prep_inputs(bert_local_out, depend, depended, no_connect,
                 depend_weight, depended_weight):
    import ml_dtypes

    x = np.ascontiguousarray(
        np.asarray(bert_local_out, dtype=np.float32).astype(ml_dtypes.bfloat16)
    )
    dep = np.asarray(depend, dtype=np.int64)
    dpd = np.asarray(depended, dtype=np.int64)
    noc = np.asarray(no_connect, dtype=np.int64)
    w1 = np.asarray(depend_weight, dtype=np.float32)
    w2 = np.asarray(depended_weight, dtype=np.float32)

    # Per-token multipliers, matching the reference's scatter order exactly.
    # Index lists hold values in [0, S); position idx+1 is affected (idx=-1
    # padding or idx=S-1 land in slots 0/S which are overwritten/cropped).
    rr = np.arange(B)[:, None]
    m1 = np.zeros((B, S + 1), np.float32)
    m1[rr, dep + 1] = 1.0
    m2 = np.zeros((B, S + 1), np.float32)
    m2[rr, dpd + 1] = 1.0
    m3 = np.zeros((B, S + 1), np.float32)
    m3[rr, dep + 1] = np.broadcast_to(w1[:, None], (B, K))
    m3[rr, dpd + 1] = np.broadcast_to(w2[:, None], (B, K))
    m3[rr, noc + 1] = 0.0
    for mm in (m1, m2, m3):
        mm[:, 0] = 1.0
    masks = np.stack([m1[:, :S], m2[:, :S], m3[:, :S]])  # [3, B, S]

    in_maps = []
    for c in range(N_CORES):
        sl = slice(c * BL, (c + 1) * BL)
        mc = masks[:, sl].reshape(3, 128, RPP)          # row = p*32 + r
        mc = np.ascontiguousarray(mc.transpose(1, 0, 2))  # [128, 3, RPP]
        in_maps.append({
            "x": x[sl].reshape(ROWS, H),
            "masks": mc.reshape(-1),
        })
    return in_maps


def kernel(bert_local_out, depend, depended, no_connect,
           depend_weight, depended_weight):
    from concourse.bass_utils import run_bass_kernel_spmd

    if "nc" not in _cache:
        _cache["nc"] = _build()
    nc = _cache["nc"]

    in_maps = _prep_inputs(bert_local_out, depend, depended, no_connect,
                           depend_weight, depended_weight)

    pdir = os.environ.get("KERNEL_PROFILE_DIR")
    ctx = contextlib.nullcontext()
    if pdir:
        import concourse.bass2jax as b2j
        from trn_agent_boot.trn_boot import _ntff_profile_via_ctypes

        if not getattr(b2j, "_neff_capture_patched", False):
            orig = b2j.rename_neff_tensors_and_patch_header

            def patched(neff_path, mapping):
                data = orig(neff_path, mapping)
                cap = os.environ.get("KERNEL_PROFILE_DIR")
                if cap:
                    os.makedirs(cap, exist_ok=True)
                    with open(os.path.join(cap, "model.neff"), "wb") as f:
                        f.write(data)
                return data

            b2j.rename_neff_tensors_and_patch_header = patched
            b2j._neff_capture_patched = True
        os.makedirs(pdir, exist_ok=True)
        hookf = _ntff_profile_via_ctypes("/opt/axon/libaxon_pjrt.so")
        if hookf is not None:
            dev = None if os.environ.get("KERNEL_PROFILE_ALL") else [0]
            ctx = hookf(pdir, dev)

    with ctx:
        res = run_bass_kernel_spmd(nc, in_maps, list(range(N_CORES)))

    outs = []
    for name in ("y1", "y2", "y3"):
        full = np.empty((B, S, H), dtype=np.float32)
        for c in range(N_CORES):
            full[c * BL : (c + 1) * BL] = (
                res.results[c][name].astype(np.float32).reshape(BL, S, H)
            )
        outs.append(full)
    return tuple(outs)


# revision 35
# speedup vs baseline: 1.1673x; 1.0156x over previous
"""Trainium2 Bass kernel for nn_DLCF_DCA (scatter_memory).

Reference computation, per sample b (B=128, S=256, H=768, K=64):
  keep_dep[s]  = (s==0) or any_k(depend[b,k] == s-1)
  keep_dpd[s]  = (s==0) or any_k(depended[b,k] == s-1)
  mult[s]      = w2 if s-1 in depended else (w1 if s-1 in depend else 0);
                 0 if s-1 in no_connect; 1 if s==0
  y1 = x * keep_dep;  y2 = x * keep_dpd;  y3 = x * mult

Strategy: pure data parallel over batch (16 samples per core, 8 cores).
The tiny per-token multiplier tables ([B, S] = 32K floats total) are
assembled on the host from the index lists; the device does the pure
memory-bound work: stream the [4096, 768] bf16 shard in (32 consecutive
token-rows per SBUF partition, so every DMA moves 6KB contiguous chunks
per partition), apply the three per-row scalars on the vector engine,
and stream the three outputs back out on three DMA queues (sync /
scalar / gpsimd) so all 16 SDMA engines stay saturated end to end.
"""

import contextlib
import os
import sys

import numpy as np

if "/opt/trn_rl_repo" not in sys.path:
    sys.path.insert(0, "/opt/trn_rl_repo")

N_CORES = 8
B, S, H, K = 128, 256, 768, 64
BL = B // N_CORES          # samples per core
ROWS = BL * S              # 4096 token-rows per core
RPP = ROWS // 128          # 32 consecutive rows per partition
NDR = 8                    # read DMA tiles (6KB/partition; completions stagger)
RPTR = RPP // NDR          # 4 row-blocks per read tile
# Write tiles per output, in row-blocks: tapered at both ends — a small
# first tile so the write stream starts as early as possible (it only needs
# the first read tile), a big middle to keep DMA counts below queue depth,
# and a small last tile so the stream end isn't bound by DVE latency.
WROWS = [4, 16, 8, 4]
WOFF = [0, 4, 20, 28]      # row offsets of each write tile

_cache = {}


def _split_multiwaits(nc, max_waits=1):
    """walrus in this container only accepts one sync-wait per instruction;
    splice extra waits onto single-wait NoOps just before the offender."""
    from concourse import mybir

    n = 0
    for func in nc.m.functions:
        for bb in func.blocks:
            insts = bb.instructions
            i = 0
            while i < len(insts):
                ins = insts[i]
                si = getattr(ins, "sync_info", None)
                if si is None or len(si.on_wait) <= max_waits:
                    i += 1
                    continue
                waits = list(si.on_wait)
                keep = waits[-max_waits:]
                extra = waits[:-max_waits]
                nops = []
                for j in range(0, len(extra), max_waits):
                    n += 1
                    nops.append(
                        mybir.InstNoOp(
                            name=f"{ins.name}-ws{n}",
                            sync_info=mybir.SyncInfo(
                                on_wait=extra[j : j + max_waits], on_update=[]
                            ),
                            bass_nofuse=True,
                            engine=ins.engine,
                            ins=[],
                            outs=[],
                        )
                    )
                si.on_wait = keep
                for k, nop in enumerate(nops):
                    insts.insert(i + k, nop)
                i += len(nops) + 1
    return n


def _build():
    import concourse.bass as bass
    import concourse.tile as tile
    from concourse import mybir

    f32 = mybir.dt.float32
    bf16 = mybir.dt.bfloat16
    mul = mybir.AluOpType.mult
    nc = bass.Bass()

    x = nc.dram_tensor("x", [ROWS, H], bf16, kind="ExternalInput")
    masks = nc.dram_tensor("masks", [128 * 3 * RPP], f32, kind="ExternalInput")
    ys = [nc.dram_tensor(f"y{i}", [ROWS, H], bf16, kind="ExternalOutput")
          for i in (1, 2, 3)]

    with tile.TileContext(nc) as tc, contextlib.ExitStack() as ctx:
        const = ctx.enter_context(tc.tile_pool(name="const", bufs=1))
        xpool = ctx.enter_context(tc.tile_pool(name="xpool", bufs=NDR))
        ypools = [
            ctx.enter_context(tc.tile_pool(name=f"y{i}p", bufs=1))
            for i in (1, 2, 3)
        ]

        # per-row multipliers, in [partition, row-in-partition] layout
        mt = const.tile([128, 3 * RPP], f32, name="masks")
        nc.sync.dma_start(out=mt[:], in_=masks.rearrange("(p c) -> p c", p=128))
        m = [mt[:, i * RPP : (i + 1) * RPP] for i in range(3)]

        # row = p*32 + r: partition p owns 32 consecutive token-rows.
        xr = x.rearrange("(p d q) h -> d p (q h)", p=128, d=NDR)
        # write view: [p, r, h] so variable-size row slices stay rectangular
        yrr = [y.rearrange("(p r) h -> p r h", p=128) for y in ys]

        # All reads on the sync queue: the scalar queue then has no read
        # backlog, so its writes can enter service the moment the gate fires
        # (a split-read layout forces BOTH queues into read-then-write FIFO
        # and re-creates a hard transition bubble).
        xts = []
        for d in range(NDR):
            t = xpool.tile([128, RPTR * H], bf16, name="xt")
            nc.sync.dma_start(out=t[:], in_=xr[d])
            xts.append(t)

        # Phase separation: pure-read burst, then pure-write burst, avoiding
        # the HBM read/write turnaround penalty (~13% per-engine throughput
        # when mixed). DVE computes freely as each x tile lands (same-queue
        # DMAs complete in FIFO order, so completions stagger). The gate
        # lives on the otherwise-idle ACT engine (gpsimd is locked out of
        # SBUF while DVE streams perf-mode ops; a DVE gate would serialize
        # all compute behind the reads): gone = x5*0.0 + 1.0 == exact 1.0,
        # dependent on the 5th of 8 reads — the last ~2.4MB of reads overlap
        # the first writes, hiding the DMA completion-receipt latency. Every
        # write tile then gets a tiny ACT "stamp" (*1.0, exact) that its
        # write issue waits on.
        copyf = mybir.ActivationFunctionType.Copy
        gone = const.tile([128, 1], f32, name="gone")
        nc.scalar.activation(gone[:], xts[2][:, :1], copyf, scale=0.0, bias=1.0)

        # queue map per (write-tile d, output yi): sync carries 6.29MB of
        # writes (plus all reads), scalar 12.58MB — equal total queue bytes.
        QMAP = {(0, 2), (1, 1), (2, 2), (3, 2)}  # -> sync; rest -> scalar
        for d, (rows, off) in enumerate(zip(WROWS, WOFF)):
            for yi in range(3):
                yt = ypools[yi].tile([128, rows * H], bf16, name=f"y{yi}d{d}")
                for g in range(rows):
                    r = off + g
                    blk = slice(g * H, (g + 1) * H)
                    nc.vector.tensor_scalar(
                        yt[:, blk], xts[r // RPTR][:, (r % RPTR) * H : (r % RPTR + 1) * H],
                        m[yi][:, r : r + 1], None, op0=mul,
                    )
                nc.scalar.activation(yt[:, :1], yt[:, :1], copyf,
                                     scale=gone[:, 0:1])
                ring = nc.sync if (d, yi) in QMAP else nc.scalar
                ring.dma_start(
                    out=yrr[yi][:, off : off + rows, :],
                    in_=yt[:].rearrange("p (r h) -> p r h", h=H),
                )

    _split_multiwaits(nc)
    return nc


def _prep_inputs(bert_local_out, depend, depended, no_connect,
                 depend_weight, depended_weight):
    import ml_dtypes

    x = np.ascontiguousarray(
        np.asarray(bert_local_out, dtype=np.float32).astype(ml_dtypes.bfloat16)
    )
    dep = np.asarray(depend, dtype=np.int64)
    dpd = np.asarray(depended, dtype=np.int64)
    noc = np.asarray(no_connect, dtype=np.int64)
    w1 = np.asarray(depend_weight, dtype=np.float32)
    w2 = np.asarray(depended_weight, dtype=np.float32)

    # Per-token multipliers, matching the reference's scatter order exactly.
    # Index lists hold values in [0, S); position idx+1 is affected (idx=-1
    # padding or idx=S-1 land in slots 0/S which are overwritten/cropped).
    rr = np.arange(B)[:, None]
    m1 = np.zeros((B, S + 1), np.float32)
    m1[rr, dep + 1] = 1.0
    m2 = np.zeros((B, S + 1), np.float32)
    m2[rr, dpd + 1] = 1.0
    m3 = np.zeros((B, S + 1), np.float32)
    m3[rr, dep + 1] = np.broadcast_to(w1[:, None], (B, K))
    m3[rr, dpd + 1] = np.broadcast_to(w2[:, None], (B, K))
    m3[rr, noc + 1] = 0.0
    for mm in (m1, m2, m3):
        mm[:, 0] = 1.0
    masks = np.stack([m1[:, :S], m2[:, :S], m3[:, :S]])  # [3, B, S]

    in_maps = []
    for c in range(N_CORES):
        sl = slice(c * BL, (c + 1) * BL)
        mc = masks[:, sl].reshape(3, 128, RPP)          # row = p*32 + r
        mc = np.ascontiguousarray(mc.transpose(1, 0, 2))  # [128, 3, RPP]
        in_maps.append({
            "x": x[sl].reshape(ROWS, H),
            "masks": mc.reshape(-1),
        })
    return in_maps


def kernel(bert_local_out, depend, depended, no_connect,
           depend_weight, depended_weight):
    from concourse.bass_utils import run_bass_kernel_spmd

    if "nc" not in _cache:
        _cache["nc"] = _build()
    nc = _cache["nc"]

    in_maps = _prep_inputs(bert_local_out, depend, depended, no_connect,
                           depend_weight, depended_weight)

    pdir = os.environ.get("KERNEL_PROFILE_DIR")
    ctx = contextlib.nullcontext()
    if pdir:
        import concourse.bass2jax as b2j
        from trn_agent_boot.trn_boot import _ntff_profile_via_ctypes

        if not getattr(b2j, "_neff_capture_patched", False):
            orig = b2j.rename_neff_tensors_and_patch_header

            def patched(neff_path, mapping):
                data = orig(neff_path, mapping)
                cap = os.environ.get("KERNEL_PROFILE_DIR")
                if cap:
                    os.makedirs(cap, exist_ok=True)
                    with open(os.path.join(cap, "model.neff"), "wb") as f:
                        f.write(data)
                return data

            b2j.rename_neff_tensors_and_patch_header = patched
            b2j._neff_capture_patched = True
        os.makedirs(pdir, exist_ok=True)
        hookf = _ntff_profile_via_ctypes("/opt/axon/libaxon_pjrt.so")
        if hookf is not None:
            dev = None if os.environ.get("KERNEL_PROFILE_ALL") else [0]
            ctx = hookf(pdir, dev)

    with ctx:
        res = run_bass_kernel_spmd(nc, in_maps, list(range(N_CORES)))

    outs = []
    for name in ("y1", "y2", "y3"):
        full = np.empty((B, S, H), dtype=np.float32)
        for c in range(N_CORES):
            full[c * BL : (c + 1) * BL] = (
                res.results[c][name].astype(np.float32).reshape(BL, S, H)
            )
        outs.append(full)
    return tuple(outs)
